# revision 8
# baseline (speedup 1.0000x reference)
"""Trainium2 Bass kernel for nn_DocREModel (DocRE relation classifier head).

Sharding: 8 cores; core c handles doc b=c//2 and pair-half h=c%2 (276 of 552
pairs). All FLOPs run on device; the host only gathers/reorders indexed data
(indices are host-visible inputs) and casts dtypes.

Per-core device pipeline (activations kept feature-major, i.e. [feat, pair]):
  A: ent_att = mean over mentions of gathered attention rows (PE matmul with
     a 0.25 one-hot matrix), bounced to DRAM (bf16).
  B: per-pair expansion of ent_att rows for head/tail entities via indirect
     DMA gathers (idx tensors are inputs).
  C: ht = sum_h(hA*tA) (DVE/Pool elementwise + tree reduce), normalize by
     row-sum (eps trick folds the 1/H), transpose via DMA-transpose, then
     rs_T = seq^T-contraction on PE.
  D: q_c_T = W_q^T rs_T; K_T = W_k^T ment_T; S_all = q_c_T^T K_T (all PE);
     mask-select + softmax over the 4 mentions (DVE/ACT); W_comb = mask *
     softmax weights; PE-transpose; pooled entity reps via one PE matmul.
  E: extractors: hs_T/ts_T = tanh(W^T [ent;rs] + b)  (PE + ACT).
  F: grouped bilinear: bl k-tiles are built as (PE one-hot row-replication of
     hs) -> (PSUM->SBUF copy) -> (DVE 2x multiply with ts), feeding a single
     PSUM-accumulated [97 x 276] classifier matmul over 384 k-tiles.
"""
import math
import os

import numpy as np
import ml_dtypes

B, L, D, H = 4, 1024, 768, 12
E, M, P = 24, 4, 552
EMB, BLK, NL = 768, 64, 97
NCORES = 8
PPC = P // 2               # pairs per core
KB = EMB // BLK            # 12 bilinear blocks
NKT = KB * 32              # 384 k-tiles of 128 in the bilinear contraction
PT_ROWS = [128, 128, PPC - 256]   # pair-partition tiles
PT_OFF = [0, 128, 256]
INV = 1.0 / math.sqrt(D)
EPS = 1e-5 * H             # eps for un-divided (by H) ht rows

BF = ml_dtypes.bfloat16


def _build_program():
    import concourse.bass as bass
    import concourse.bacc as bacc
    import concourse.mybir as mybir
    from concourse.tile import TileContext
    from concourse.masks import make_identity

    bf16 = mybir.dt.bfloat16
    f32 = mybir.dt.float32
    u32 = mybir.dt.uint32
    AF = mybir.ActivationFunctionType
    OP = mybir.AluOpType

    nc = bacc.Bacc("TRN2", target_bir_lowering=False, debug=False)

    # ---- I/O ----
    matt = nc.dram_tensor("matt", [E * M, L * H], bf16, kind="ExternalInput")
    seq = nc.dram_tensor("seq", [L, D], bf16, kind="ExternalInput")
    mct = nc.dram_tensor("mct", [D, E * M], bf16, kind="ExternalInput")
    mc = nc.dram_tensor("mc", [E * M, D], bf16, kind="ExternalInput")
    wq = nc.dram_tensor("wq", [D, D], bf16, kind="ExternalInput")
    wk = nc.dram_tensor("wk", [D, D], bf16, kind="ExternalInput")
    hw = nc.dram_tensor("hw", [2 * D, D], bf16, kind="ExternalInput")
    tw = nc.dram_tensor("tw", [2 * D, D], bf16, kind="ExternalInput")
    hb = nc.dram_tensor("hb", [D, 1], f32, kind="ExternalInput")
    tb = nc.dram_tensor("tb", [D, 1], f32, kind="ExternalInput")
    bw = nc.dram_tensor("bw", [2, 128, NKT // 2, NL], bf16, kind="ExternalInput")
    bb = nc.dram_tensor("bb", [NL, 1], f32, kind="ExternalInput")
    mavg = nc.dram_tensor("mavg", [E * M, E], bf16, kind="ExternalInput")
    rep = nc.dram_tensor("rep", [BLK, 32, 128], bf16, kind="ExternalInput")
    idxh = nc.dram_tensor("idxh", [3, 128, 1], u32, kind="ExternalInput")
    idxt = nc.dram_tensor("idxt", [3, 128, 1], u32, kind="ExternalInput")
    mkh = nc.dram_tensor("mkh", [3, 128, E * M], bf16, kind="ExternalInput")
    mkt = nc.dram_tensor("mkt", [3, 128, E * M], bf16, kind="ExternalInput")
    out = nc.dram_tensor("out", [NL, PPC], f32, kind="ExternalOutput")

    entA_dram = nc.dram_tensor("entA_scratch", [E, L * H], bf16)

    with TileContext(nc) as tc:
        with tc.tile_pool(name="keep", bufs=1) as kp:
            ident = kp.tile([128, 128], bf16, tag="ident")
            make_identity(nc, ident[:])

            # persistent activation tiles (feature-major)
            rs_T = kp.tile([128, 6, PPC], bf16, tag="rs_T")
            qc_T = kp.tile([128, 6, PPC], bf16, tag="qc_T")
            kproj_T = kp.tile([128, 6, E * M], bf16, tag="kproj_T")
            hent_T = kp.tile([128, 6, PPC], bf16, tag="hent_T")
            tent_T = kp.tile([128, 6, PPC], bf16, tag="tent_T")
            hs_T = kp.tile([128, 6, PPC], bf16, tag="hs_T")
            ts_T = kp.tile([128, 6, PPC], bf16, tag="ts_T")
            htn_T = kp.tile([128, 8, 288], bf16, tag="htn_T")  # 288 = 276 padded for 16-div transpose
            wcombh_T = kp.tile([E * M, PPC], bf16, tag="wcombh_T")
            wcombt_T = kp.tile([E * M, PPC], bf16, tag="wcombt_T")

            bb_sb = kp.tile([NL, 1], f32, tag="bb")
            nc.sync.dma_start(out=bb_sb[:], in_=bb[:, :])
            hb_sb = kp.tile([128, 6, 1], f32, tag="hb")
            nc.sync.dma_start(out=hb_sb[:], in_=hb.rearrange("(a p) o -> p a o", p=128))
            tb_sb = kp.tile([128, 6, 1], f32, tag="tb")
            nc.sync.dma_start(out=tb_sb[:], in_=tb.rearrange("(a p) o -> p a o", p=128))

            mask_sb = {}
            for s, srcm in (("h", mkh), ("t", mkt)):
                for pt in range(3):
                    t = kp.tile([128, E * M], bf16, tag=f"mk{s}{pt}")
                    nc.sync.dma_start(out=t[:PT_ROWS[pt]], in_=srcm[pt, :PT_ROWS[pt], :])
                    mask_sb[(s, pt)] = t
            idx_sb = {}
            for s, srci in (("h", idxh), ("t", idxt)):
                for pt in range(3):
                    t = kp.tile([128, 1], u32, tag=f"idx{s}{pt}")
                    nc.sync.dma_start(out=t[:PT_ROWS[pt]], in_=srci[pt, :PT_ROWS[pt], :])
                    idx_sb[(s, pt)] = t

            # ---------------- phase A: entity attention means ----------------
            with (
                tc.tile_pool(name="pA", bufs=1) as pA,
                tc.tile_pool(name="psA", bufs=2, space="PSUM") as psA,
            ):
                matt_sb = pA.tile([E * M, L * H], bf16, tag="matt")
                nc.sync.dma_start(out=matt_sb[:], in_=matt[:, :])
                mavg_sb = pA.tile([E * M, E], bf16, tag="mavg")
                nc.sync.dma_start(out=mavg_sb[:], in_=mavg[:, :])
                entA_sb = pA.tile([E, L * H], bf16, tag="entA")
                for c6 in range(6):
                    ps = psA.tile([E, 2048], mybir.dt.float32, tag="psA")
                    for q in range(4):
                        nc.tensor.matmul(
                            out=ps[:, q * 512:(q + 1) * 512],
                            lhsT=mavg_sb[:],
                            rhs=matt_sb[:, c6 * 2048 + q * 512: c6 * 2048 + (q + 1) * 512],
                            start=True, stop=True,
                        )
                    eng = nc.scalar if c6 % 2 == 0 else nc.vector
                    if c6 % 2 == 0:
                        nc.scalar.copy(out=entA_sb[:, c6 * 2048:(c6 + 1) * 2048], in_=ps[:])
                    else:
                        nc.vector.tensor_copy(out=entA_sb[:, c6 * 2048:(c6 + 1) * 2048], in_=ps[:])
                nc.sync.dma_start(out=entA_dram[:, :], in_=entA_sb[:])

            # ---------------- phases B+C: per-pair ht rows + rs_T ----------------
            seq_sb = kp.tile([128, 8, D], bf16, tag="seq")
            nc.sync.dma_start(out=seq_sb[:], in_=seq.rearrange("(a p) d -> p a d", p=128))

            psS_cm = tc.tile_pool(name="psS", bufs=2, space="PSUM")
            psS = psS_cm.__enter__()
            with (
                tc.tile_pool(name="gath", bufs=4) as pg,
                tc.tile_pool(name="pC", bufs=2) as pC,
            ):
                for pt in range(3):
                    rows = PT_ROWS[pt]
                    hA = pg.tile([128, L * H], bf16, tag="gath")
                    tA = pg.tile([128, L * H], bf16, tag="gath")
                    nc.gpsimd.indirect_dma_start(
                        out=hA[:rows], out_offset=None, in_=entA_dram[:, :],
                        in_offset=bass.IndirectOffsetOnAxis(ap=idx_sb[("h", pt)][:rows, :1], axis=0),
                    )
                    nc.gpsimd.indirect_dma_start(
                        out=tA[:rows], out_offset=None, in_=entA_dram[:, :],
                        in_offset=bass.IndirectOffsetOnAxis(ap=idx_sb[("t", pt)][:rows, :1], axis=0),
                    )
                    # products: hA *= tA, viewing [p, l, h]
                    prod = hA
                    peng = nc.vector if pt != 1 else nc.gpsimd
                    peng.tensor_tensor(out=prod[:rows], in0=hA[:rows], in1=tA[:rows], op=OP.mult)
                    # tree-reduce over h (h is the inner dim of (l, h))
                    p3 = prod[:rows].rearrange("p (l h) -> p l h", h=H)
                    nc.vector.tensor_tensor(out=p3[:, :, 0:6], in0=p3[:, :, 0:6], in1=p3[:, :, 6:12], op=OP.add)
                    nc.vector.tensor_tensor(out=p3[:, :, 0:3], in0=p3[:, :, 0:3], in1=p3[:, :, 3:6], op=OP.add)
                    tsum = pC.tile([128, L], bf16, tag="tsum")
                    nc.vector.tensor_tensor(out=tsum[:rows], in0=p3[:, :, 0], in1=p3[:, :, 1], op=OP.add)
                    ht = pC.tile([128, L], mybir.dt.float32, tag="ht")
                    nc.vector.tensor_tensor(out=ht[:rows], in0=tsum[:rows], in1=p3[:, :, 2], op=OP.add)
                    # normalize: r = 1/(sum + H*1e-5); htn = ht*r (bf16)
                    ssum = pC.tile([128, 1], mybir.dt.float32, tag="ssum")
                    nc.vector.tensor_reduce(out=ssum[:rows], in_=ht[:rows], axis=mybir.AxisListType.X, op=OP.add)
                    nc.vector.tensor_scalar(out=ssum[:rows], in0=ssum[:rows], scalar1=EPS, scalar2=None, op0=OP.add)
                    nc.vector.reciprocal(out=ssum[:rows], in_=ssum[:rows])
                    htn = pC.tile([128, L], bf16, tag="htn")
                    trows = rows if rows % 16 == 0 else (rows + 15) // 16 * 16
                    if trows != rows:
                        nc.vector.memset(htn[0:trows], 0.0)
                    nc.vector.tensor_scalar(out=htn[:rows], in0=ht[:rows], scalar1=ssum[:rows, :1], scalar2=None, op0=OP.mult)
                    for lt in range(8):
                        nc.sync.dma_start_transpose(
                            htn_T[:, lt, PT_OFF[pt]:PT_OFF[pt] + trows],
                            htn[:trows, lt * 128:(lt + 1) * 128],
                        )

                # rs_T[dm] = sum_lt seq[lt,dm]^T-contract htn_T[lt]
                for dm in range(6):
                    ps = psS.tile([128, PPC], mybir.dt.float32, tag="psC")
                    for lt in range(8):
                        nc.tensor.matmul(
                            out=ps[:],
                            lhsT=seq_sb[:, lt, dm * 128:(dm + 1) * 128],
                            rhs=htn_T[:, lt, :PPC],
                            start=(lt == 0), stop=(lt == 7),
                        )
                    nc.scalar.copy(out=rs_T[:, dm, :], in_=ps[:])

            # ---------------- phase D: attention pooling ----------------
            with tc.tile_pool(name="pD", bufs=1) as pD:
                wq_sb = pD.tile([128, 6, D], bf16, tag="wq")
                nc.sync.dma_start(out=wq_sb[:], in_=wq.rearrange("(a p) d -> p a d", p=128))
                wk_sb = pD.tile([128, 6, D], bf16, tag="wk")
                nc.sync.dma_start(out=wk_sb[:], in_=wk.rearrange("(a p) d -> p a d", p=128))
                mct_sb = pD.tile([128, 6, E * M], bf16, tag="mct")
                nc.sync.dma_start(out=mct_sb[:], in_=mct.rearrange("(a p) d -> p a d", p=128))
                mc_sb = pD.tile([E * M, D], bf16, tag="mc")
                nc.sync.dma_start(out=mc_sb[:], in_=mc[:, :])

                for dm in range(6):
                    ps = psS.tile([128, PPC], mybir.dt.float32, tag="psC")
                    for kt in range(6):
                        nc.tensor.matmul(
                            out=ps[:], lhsT=wq_sb[:, kt, dm * 128:(dm + 1) * 128],
                            rhs=rs_T[:, kt, :], start=(kt == 0), stop=(kt == 5),
                        )
                    nc.scalar.copy(out=qc_T[:, dm, :], in_=ps[:])
                for dm in range(6):
                    ps = psS.tile([128, E * M], mybir.dt.float32, tag="psD")
                    for kt in range(6):
                        nc.tensor.matmul(
                            out=ps[:], lhsT=wk_sb[:, kt, dm * 128:(dm + 1) * 128],
                            rhs=mct_sb[:, kt, :], start=(kt == 0), stop=(kt == 5),
                        )
                    nc.scalar.copy(out=kproj_T[:, dm, :], in_=ps[:])

                for pt in range(3):
                    rows = PT_ROWS[pt]
                    ps_sall = psS.tile([128, E * M], mybir.dt.float32, tag="psD")
                    for kt in range(6):
                        nc.tensor.matmul(
                            out=ps_sall[:rows],
                            lhsT=qc_T[:, kt, PT_OFF[pt]:PT_OFF[pt] + rows],
                            rhs=kproj_T[:, kt, :], start=(kt == 0), stop=(kt == 5),
                        )
                    for s, wcomb_T in (("h", wcombh_T), ("t", wcombt_T)):
                        msk = mask_sb[(s, pt)]
                        with tc.tile_pool(name="pDs", bufs=1) as pDs:
                            tmp = pDs.tile([128, E * M], mybir.dt.float32, tag="tmp")
                            nc.vector.tensor_tensor(out=tmp[:rows], in0=ps_sall[:rows], in1=msk[:rows], op=OP.mult)
                            ssel = pDs.tile([128, M], mybir.dt.float32, tag="ssel")
                            nc.vector.tensor_reduce(
                                out=ssel[:rows],
                                in_=tmp[:rows].rearrange("p (e m) -> p m e", m=M),
                                axis=mybir.AxisListType.X, op=OP.add,
                            )
                            mx = pDs.tile([128, 1], mybir.dt.float32, tag="mx")
                            nc.vector.tensor_reduce(out=mx[:rows], in_=ssel[:rows], axis=mybir.AxisListType.X, op=OP.max)
                            nc.vector.tensor_scalar(out=mx[:rows], in0=mx[:rows], scalar1=-INV, scalar2=None, op0=OP.mult)
                            wexp = pDs.tile([128, M], mybir.dt.float32, tag="wexp")
                            nc.scalar.activation(out=wexp[:rows], in_=ssel[:rows], func=AF.Exp,
                                                 bias=mx[:rows, :1], scale=INV)
                            sse = pDs.tile([128, 1], mybir.dt.float32, tag="sse")
                            nc.vector.tensor_reduce(out=sse[:rows], in_=wexp[:rows], axis=mybir.AxisListType.X, op=OP.add)
                            nc.vector.reciprocal(out=sse[:rows], in_=sse[:rows])
                            wcomb = pDs.tile([128, E * M], bf16, tag="wcomb")
                            nc.vector.scalar_tensor_tensor(
                                out=wcomb[:rows],
                                in0=wexp[:rows, None, :].broadcast_to([rows, E, M]),
                                scalar=sse[:rows, :1],
                                in1=msk[:rows].rearrange("p (e m) -> p e m", m=M),
                                op0=OP.mult, op1=OP.mult,
                            )
                            ps_t = psS.tile([E * M, 128], bf16, tag="psDt")
                            nc.tensor.transpose(out=ps_t[:, :rows], in_=wcomb[:rows], identity=ident[:rows, :rows])
                            nc.scalar.copy(out=wcomb_T[:, PT_OFF[pt]:PT_OFF[pt] + rows], in_=ps_t[:, :rows])

                for wcomb_T, ent_T in ((wcombh_T, hent_T), (wcombt_T, tent_T)):
                    for dm in range(6):
                        ps = psS.tile([128, PPC], mybir.dt.float32, tag="psC")
                        nc.tensor.matmul(
                            out=ps[:], lhsT=mc_sb[:, dm * 128:(dm + 1) * 128],
                            rhs=wcomb_T[:], start=True, stop=True,
                        )
                        nc.scalar.copy(out=ent_T[:, dm, :], in_=ps[:])

            # ---------------- phase E: extractors ----------------
            with tc.tile_pool(name="pE", bufs=1) as pE:
                hw_sb = pE.tile([128, 12, D], bf16, tag="hw")
                nc.sync.dma_start(out=hw_sb[:], in_=hw.rearrange("(a p) d -> p a d", p=128))
                tw_sb = pE.tile([128, 12, D], bf16, tag="tw")
                nc.sync.dma_start(out=tw_sb[:], in_=tw.rearrange("(a p) d -> p a d", p=128))
                for w_sb, ent_T, bias_sb, o_T in (
                    (hw_sb, hent_T, hb_sb, hs_T),
                    (tw_sb, tent_T, tb_sb, ts_T),
                ):
                    for dm in range(6):
                        ps = psS.tile([128, PPC], mybir.dt.float32, tag="psC")
                        for kt in range(12):
                            rhs = ent_T[:, kt, :] if kt < 6 else rs_T[:, kt - 6, :]
                            nc.tensor.matmul(
                                out=ps[:], lhsT=w_sb[:, kt, dm * 128:(dm + 1) * 128],
                                rhs=rhs, start=(kt == 0), stop=(kt == 11),
                            )
                        nc.scalar.activation(out=o_T[:, dm, :], in_=ps[:], func=AF.Tanh,
                                             bias=bias_sb[:, dm, :1], scale=1.0)

            # ---------------- phase F: grouped bilinear classifier ----------------
            psS_cm.__exit__(None, None, None)
            with (
                tc.tile_pool(name="pF", bufs=1) as pF,
                tc.tile_pool(name="pFs", bufs=6) as pFs,
                tc.tile_pool(name="psF", bufs=4, space="PSUM") as psF,
                tc.tile_pool(name="psL", bufs=1, space="PSUM") as psL,
            ):
                bw_sb = []
                for hh in range(2):
                    bwt = pF.tile([128, NKT // 2, NL], bf16, tag=f"bw{hh}")
                    bw_sb.append(bwt)
                    nc.sync.dma_start(out=bwt[:], in_=bw[hh, :, :, :])
                rep_sb = pF.tile([BLK, 32, 128], bf16, tag="rep")
                nc.sync.dma_start(out=rep_sb[:], in_=rep[:, :, :])
                # hs2: [64, kb, PPC] re-chunk of hs_T; ts2: [128, kb, PPC] doubled ts rows
                hs2 = pF.tile([BLK, KB, PPC], bf16, tag="hs2")
                ts2 = pF.tile([128, KB, PPC], bf16, tag="ts2")
                for kb in range(KB):
                    src = hs_T[(kb % 2) * 64:(kb % 2) * 64 + 64, kb // 2, :]
                    nc.vector.tensor_copy(out=hs2[:, kb, :], in_=src)
                    srct = ts_T[(kb % 2) * 64:(kb % 2) * 64 + 64, kb // 2, :]
                    nc.vector.tensor_copy(out=ts2[0:64, kb, :], in_=srct)
                    nc.vector.tensor_copy(out=ts2[64:128, kb, :], in_=srct)

                ps_log = psL.tile([NL, PPC], mybir.dt.float32, tag="pslog")
                for kt in range(NKT):
                    kb, ip = kt // 32, kt % 32
                    ps_rep = psF.tile([128, PPC], mybir.dt.float32, tag="psrep")
                    nc.tensor.matmul(out=ps_rep[:], lhsT=rep_sb[:, ip, :], rhs=hs2[:, kb, :],
                                     start=True, stop=True)
                    hsrep = pFs.tile([128, PPC], bf16, tag="hsrep")
                    if kt % 2 == 0:
                        nc.scalar.copy(out=hsrep[:], in_=ps_rep[:])
                    else:
                        nc.vector.tensor_copy(out=hsrep[:], in_=ps_rep[:])
                    bl = pFs.tile([128, PPC], bf16, tag="bl")
                    beng = nc.gpsimd if kt % 3 == 2 else nc.vector
                    beng.tensor_tensor(out=bl[:], in0=ts2[:, kb, :], in1=hsrep[:], op=OP.mult)
                    nc.tensor.matmul(
                        out=ps_log[:],
                        lhsT=bw_sb[kt // (NKT // 2)][:, kt % (NKT // 2), :],
                        rhs=bl[:],
                        start=(kt == 0), stop=(kt == NKT - 1),
                    )
                out_sb = pF.tile([NL, PPC], mybir.dt.float32, tag="outsb")
                nc.scalar.activation(out=out_sb[:], in_=ps_log[:], func=AF.Identity,
                                     bias=bb_sb[:, :1], scale=1.0)
                nc.sync.dma_start(out=out[:, :], in_=out_sb[:])

    nc.finalize()
    return nc


def _host_inputs(inputs):
    """Build the 8 per-core input maps from the full problem inputs."""
    seq = np.asarray(inputs["sequence_output"], np.float32)
    att = np.asarray(inputs["attention"], np.float32)
    W_q = np.asarray(inputs["W_q"], np.float32)
    W_k = np.asarray(inputs["W_k"], np.float32)
    head_W = np.asarray(inputs["head_W"], np.float32)
    head_b = np.asarray(inputs["head_b"], np.float32)
    tail_W = np.asarray(inputs["tail_W"], np.float32)
    tail_b = np.asarray(inputs["tail_b"], np.float32)
    bil_W = np.asarray(inputs["bil_W"], np.float32)
    bil_b = np.asarray(inputs["bil_b"], np.float32)
    mention_pos = np.asarray(inputs["mention_pos"])
    hts = np.asarray(inputs["hts"])

    bw = np.ascontiguousarray(
        bil_W.reshape(2, NKT // 2, 128, NL).transpose(0, 2, 1, 3)
    ).astype(BF)
    mavg = np.zeros((E * M, E), np.float32)
    for e in range(E):
        mavg[e * M:(e + 1) * M, e] = 0.25
    mavg = mavg.astype(BF)
    rep = np.zeros((BLK, 32, 128), np.float32)
    for ip in range(32):
        for q in range(128):
            rep[2 * ip + q // 64, ip, q] = 1.0
    rep = rep.astype(BF)

    shared = dict(
        wq=W_q.astype(BF), wk=W_k.astype(BF),
        hw=head_W.astype(BF), tw=tail_W.astype(BF),
        hb=head_b.reshape(D, 1).copy(), tb=tail_b.reshape(D, 1).copy(),
        bw=bw, bb=bil_b.reshape(NL, 1).copy(), mavg=mavg, rep=rep,
    )

    in_maps = []
    for c in range(NCORES):
        b = c // 2
        sl = slice((c % 2) * PPC, (c % 2) * PPC + PPC)
        pos = mention_pos[b] + 1
        pf = pos.reshape(-1).astype(np.int64)
        matt = np.ascontiguousarray(
            att[b][:, pf, :].transpose(1, 2, 0).reshape(E * M, L * H)
        ).astype(BF)
        ment_c = seq[b][pf]                       # [96, 768]
        e1 = hts[b, sl, 0].astype(np.int64)
        e2 = hts[b, sl, 1].astype(np.int64)

        def pad_idx(e_idx):
            o = np.zeros((3, 128, 1), np.uint32)
            for pt in range(3):
                r = PT_ROWS[pt]
                o[pt, :r, 0] = e_idx[PT_OFF[pt]:PT_OFF[pt] + r]
            return o

        def masks(e_idx):
            o = np.zeros((3, 128, E * M), np.float32)
            for pt in range(3):
                r = PT_ROWS[pt]
                seg = e_idx[PT_OFF[pt]:PT_OFF[pt] + r]
                for i, e in enumerate(seg):
                    o[pt, i, e * M:(e + 1) * M] = 1.0
            return o.astype(BF)

        in_maps.append(dict(
            matt=matt,
            seq=seq[b].astype(BF),
            mct=np.ascontiguousarray(ment_c.T).astype(BF),
            mc=ment_c.astype(BF),
            idxh=pad_idx(e1), idxt=pad_idx(e2),
            mkh=masks(e1), mkt=masks(e2),
            **shared,
        ))
    return in_maps


_CACHE = {}


def kernel(**inputs):
    from concourse.bass_utils import run_bass_kernel_spmd

    in_maps = _host_inputs(inputs)
    if "nc" not in _CACHE:
        _CACHE["nc"] = _build_program()
    nc = _CACHE["nc"]
    res = run_bass_kernel_spmd(nc, in_maps, core_ids=list(range(NCORES)))
    full = np.zeros((B, P, NL), np.float32)
    for c in range(NCORES):
        b = c // 2
        sl = slice((c % 2) * PPC, (c % 2) * PPC + PPC)
        full[b, sl] = np.asarray(res.results[c]["out"]).T
    return full.reshape(B * P, NL)
